# revision 11
# baseline (speedup 1.0000x reference)
"""v2 Bass/Trainium2 kernel for nn_CF_87866440942102 — banked dma_gather design.

SPMD over 8 cores, tables replicated, no collectives. Uniform (core-independent)
gather schedules: fixed per-bank capacities, padded with repeated indices.

  Stream S0 (per core): its 16384 owned side-0 uniques, grouped by 32768-row
    bank of xu0[j], each bank padded to a fixed 768-row capacity; gathered from
    the entity table and a host-padded bias table via dma_gather (int16 local
    idx); computes kl_bias/kl_entity and writes emb0 rows
    [ent_mean(32), bias_mean, 1, ...] to DRAM scratch.
  Stream S1 (identical on all cores): ALL 65536 side-1 uniques, value-sorted
    and banked exactly (same schedule everywhere since the data is shared);
    emb1 rows are [ent_mean(32), 1, bias_mean] so that
    dot(emb0_row[0:34], emb1_row[0:34]) = ent dot + bias0 + bias1.
    Side-1 KL results are taken from core 0 at assembly.
  Phase B: batch rerouted so core c handles elements with x0//16384 == c,
    sorted by emb1-rank bank with fixed per-bank tile capacities; per 2048-
    element tile, 2+2 dma_gathers fetch emb0[rank0[x0]] / emb1[rank1[x1]] and
    pred = mgb + sum(e0[0:34]*e1[0:34]).

All model-data gathers run on device; the host does index bookkeeping only.
"""

import sys

if "/opt/trn_rl_repo" not in sys.path:
    sys.path.insert(0, "/opt/trn_rl_repo")

import numpy as np

import concourse.bass as bass
import concourse.bacc as bacc
import concourse.mybir as mybir
import concourse.tile as tile
from concourse.bass_utils import run_bass_kernel_spmd

P = 128
F32 = mybir.dt.float32
I16 = mybir.dt.int16

N, M = 1_000_000, 200_000
E = 32
B = 262144
U1, U2 = 131072, 65536
N_CORES = 8
U1C = U1 // N_CORES   # 16384
NROWS = N + M

BANK = 32768
CHUNK = 1024
TROWS = 4096          # phase-A tile rows; dest [128, 32, 64]
BT = 2048             # phase-B elements per tile
S0CAP = 768           # per-bank capacity for S0 (E[count]=537, sigma~23)
NB_TAB = -(-NROWS // BANK)  # 37 banks over the parameter tables


def _schedule_s0():
    """Uniform S0 schedule: bank b of the table at stream rows
    [b*S0CAP, (b+1)*S0CAP). Returns (instrs [(bank, r0, nrows)], s0rows)."""
    nb = -(-N // BANK)  # side-0 values < N -> 31 banks
    rows = nb * S0CAP
    s0rows = -(-rows // TROWS) * TROWS
    instrs = []
    for b in range(nb):
        r = b * S0CAP
        while r < (b + 1) * S0CAP:
            nxt = min(((r // CHUNK) + 1) * CHUNK, (b + 1) * S0CAP)
            # also stay within a TROWS tile (CHUNK divides TROWS so ok)
            instrs.append((b, r, nxt - r))
            r = nxt
    # tail pad rows reuse the last bank
    r = rows
    while r < s0rows:
        nxt = min(((r // CHUNK) + 1) * CHUNK, s0rows)
        instrs.append((nb - 1, r, nxt - r))
        r = nxt
    return instrs, s0rows


def _pack_s0(vals):
    """vals: xu0 slice [16384]. Returns (idx16 [s0rows], rank [16384])."""
    instrs, s0rows = _schedule_s0()
    nb = -(-N // BANK)
    idx16 = np.zeros(s0rows, np.int16)
    rank = np.empty(len(vals), np.int64)
    order = np.argsort(vals, kind="stable")
    sv = vals[order]
    bounds = np.searchsorted(sv, np.arange(nb + 1) * BANK)
    for b in range(nb):
        i, j = bounds[b], bounds[b + 1]
        cnt = j - i
        assert cnt <= S0CAP, f"bank {b} count {cnt} > cap {S0CAP}"
        r0 = b * S0CAP
        rank[order[i:j]] = r0 + np.arange(cnt)
        seg = (sv[i:j] % BANK).astype(np.int16)
        idx16[r0:r0 + cnt] = seg
        fill = seg[-1] if cnt else np.int16(0)
        idx16[r0 + cnt:r0 + S0CAP] = fill
    idx16[nb * S0CAP:] = idx16[nb * S0CAP - 1]
    return idx16, rank


def _plan_s1(vals):
    """Exact value-sorted banked plan for the shared side-1 stream."""
    n = len(vals)
    order = np.argsort(vals, kind="stable")
    sv = vals[order]
    idx_parts, instrs = [], []
    rank = np.empty(n, np.int64)
    out = 0
    i = 0
    while i < n:
        b = int(sv[i]) // BANK
        j = i
        while j < n and int(sv[j]) // BANK == b:
            j += 1
        cnt = j - i
        pad = (-cnt) % 128
        rank[order[i:j]] = out + np.arange(cnt)
        idx_parts.append((sv[i:j] % BANK).astype(np.int16))
        if pad:
            idx_parts.append(np.full(pad, idx_parts[-1][-1], np.int16))
        r = out
        while r < out + cnt + pad:
            nxt = min(((r // CHUNK) + 1) * CHUNK, out + cnt + pad)
            instrs.append((b, r, nxt - r))
            r = nxt
        out += cnt + pad
        i = j
    s1rows = -(-out // TROWS) * TROWS
    if s1rows > out:
        idx_parts.append(np.full(s1rows - out, idx_parts[-1][-1], np.int16))
        b_last = int(sv[-1]) // BANK
        r = out
        while r < s1rows:
            nxt = min(((r // CHUNK) + 1) * CHUNK, s1rows)
            instrs.append((b_last, r, nxt - r))
            r = nxt
    return np.concatenate(idx_parts), instrs, rank


def wrap16(idx16):
    a = idx16.reshape(-1, 16).T
    return np.tile(a, (8, 1)).astype(np.int16)


def _phaseb_caps(s1rows):
    """Per-rank-bank tile capacities (in BT tiles), uniform across cores."""
    nb1 = -(-s1rows // BANK)
    caps = []
    for b in range(nb1):
        # expected elements per core hitting rank-bank b (real ranks ~U2)
        lo, hi = b * BANK, min((b + 1) * BANK, s1rows)
        width = max(0, min(hi, U2 * 2) - lo)  # ranks ~ uniform over realrows
        exp = (B // N_CORES) * width / max(s1rows, 1)
        cap_elems = int(exp + 6 * np.sqrt(max(exp, 1)) + 512)
        caps.append(-(-cap_elems // BT))
    return caps


def build_nc_v2(n_cores, s0rows, s1rows, instrs0, instrs1, tile_banks, e=E,
                bufs=2):
    nc = bacc.Bacc("TRN2", target_bir_lowering=False, debug=False,
                   num_devices=n_cores)
    ap = lambda t: t.ap()
    e1 = ap(nc.dram_tensor("e1", [NROWS, 64], F32, kind="ExternalInput"))
    bp = ap(nc.dram_tensor("bp64", [NROWS, 64], F32, kind="ExternalInput"))
    ntb = len(tile_banks)
    nt0 = s0rows // TROWS
    nt1 = s1rows // TROWS
    idxA = ap(nc.dram_tensor("idxA", [P, (s0rows + s1rows) // 16], I16,
                             kind="ExternalInput"))
    idxB = ap(nc.dram_tensor("idxB", [P, (ntb * BT * 2) // 16], I16,
                             kind="ExternalInput"))
    mgb = ap(nc.dram_tensor("mgb", [P, 1], F32, kind="ExternalInput"))
    baux = ap(nc.dram_tensor("baux", [2, P, 3], F32, kind="ExternalInput"))
    eaux = ap(nc.dram_tensor("eaux", [2, 3, P, 1, e], F32, kind="ExternalInput"))
    klb = ap(nc.dram_tensor("klb", [nt0 + nt1, P, TROWS // P], F32,
                            kind="ExternalOutput"))
    kle = ap(nc.dram_tensor("kle", [nt0 + nt1, P, TROWS // P, e], F32,
                            kind="ExternalOutput"))
    pred = ap(nc.dram_tensor("pred", [ntb, P, BT // P], F32,
                             kind="ExternalOutput"))

    TT = mybir.AluOpType
    TC = TROWS // P

    with tile.TileContext(nc) as tc:
        with (
            tc.tile_pool(name="consts", bufs=1) as cp,
            tc.tile_pool(name="dram", bufs=1, space="DRAM") as dp,
            tc.tile_pool(name="work", bufs=bufs) as pool,
        ):
            emb0 = dp.tile([s0rows, 64], F32, tag="emb0")
            emb1 = dp.tile([s1rows, 64], F32, tag="emb1")

            mgb_t = cp.tile([P, 1], F32, tag="mgb")
            nc.sync.dma_start(out=mgb_t[:], in_=mgb[:, :])
            idxA_t = cp.tile([P, (s0rows + s1rows) // 16], I16, tag="idxA")
            nc.sync.dma_start(out=idxA_t[:], in_=idxA[:, :])
            idxB_t = cp.tile([P, (ntb * BT * 2) // 16], I16, tag="idxB")
            nc.sync.dma_start(out=idxB_t[:], in_=idxB[:, :])
            baux_t = []
            for g in range(2):
                t_ = cp.tile([P, 3], F32, tag=f"baux{g}")
                nc.sync.dma_start(out=t_[:], in_=baux[g])
                baux_t.append(t_)
            eaux_t = []
            for g in range(2):
                row = []
                for j in range(3):
                    t_ = cp.tile([P, 1, e], F32, tag=f"eaux{g}{j}")
                    nc.sync.dma_start(out=t_[:], in_=eaux[g, j])
                    row.append(t_)
                eaux_t.append(row)

            streams = [
                (0, s0rows, instrs0, 0, emb0, 0),
                (1, s1rows, instrs1, s0rows, emb1, 1),
            ]
            tglob = 0
            for g, srows, instrs, idx_off, embd, side in streams:
                ntile = srows // TROWS
                per_tile = [[] for _ in range(ntile)]
                for (b, r0, nr) in instrs:
                    per_tile[r0 // TROWS].append((b, r0, nr))
                for t_i in range(ntile):
                    ent = pool.tile([P, TC, 64], F32, tag="ent")
                    bia = pool.tile([P, TC, 64], F32, tag="bia")
                    for (b, r0, nr) in per_tile[t_i]:
                        base = b * BANK
                        hi = min(base + BANK, NROWS)
                        col0 = (r0 % TROWS) // P
                        isl = idxA_t[:, (idx_off + r0) // 16:
                                     (idx_off + r0 + nr) // 16]
                        nc.gpsimd.dma_gather(
                            out_ap=ent[:, col0:col0 + nr // P, :],
                            in_ap=e1[base:hi, :], idxs_ap=isl,
                            num_idxs=nr, num_idxs_reg=nr, elem_size=64,
                        )
                        nc.gpsimd.dma_gather(
                            out_ap=bia[:, col0:col0 + nr // P, :],
                            in_ap=bp[base:hi, :], idxs_ap=isl,
                            num_idxs=nr, num_idxs_reg=nr, elem_size=64,
                        )
                    em = ent[:, :, 0:e]
                    es = ent[:, :, e:2 * e]
                    bm = bia[:, :, 0:1]
                    bs = bia[:, :, 1:2]
                    mp_b = baux_t[g][:, 0:1].to_broadcast([P, TC, 1])
                    c0_b = baux_t[g][:, 1:2].to_broadcast([P, TC, 1])
                    c1_b = baux_t[g][:, 2:3].to_broadcast([P, TC, 1])
                    mp_e = eaux_t[g][0][:].to_broadcast([P, TC, e])
                    c0_e = eaux_t[g][1][:].to_broadcast([P, TC, e])
                    c1_e = eaux_t[g][2][:].to_broadcast([P, TC, e])

                    emb_t = pool.tile([P, TC, 64], F32, tag="embt")
                    nc.vector.tensor_copy(out=emb_t[:, :, 0:e], in_=em)
                    if side == 0:
                        nc.vector.tensor_copy(out=emb_t[:, :, e:e + 1], in_=bm)
                        nc.vector.memset(emb_t[:, :, e + 1:e + 2], 1.0)
                    else:
                        nc.vector.memset(emb_t[:, :, e:e + 1], 1.0)
                        nc.vector.tensor_copy(out=emb_t[:, :, e + 1:e + 2], in_=bm)
                    nc.sync.dma_start(
                        out=embd[t_i * TROWS:(t_i + 1) * TROWS, :].rearrange(
                            "(c p) d -> p c d", p=P),
                        in_=emb_t[:],
                    )

                    lnb = pool.tile([P, TC, 1], F32, tag="lnb")
                    nc.vector.tensor_tensor(out=bs, in0=bs, in1=bs, op=TT.mult)
                    nc.scalar.activation(
                        out=lnb[:], in_=bs,
                        func=mybir.ActivationFunctionType.Ln)
                    nc.vector.tensor_tensor(out=bm, in0=bm, in1=mp_b,
                                            op=TT.subtract)
                    nc.vector.tensor_tensor(out=bm, in0=bm, in1=bm, op=TT.mult)
                    nc.vector.tensor_tensor(out=bm, in0=bm, in1=bs, op=TT.add)
                    nc.vector.tensor_tensor(out=bm, in0=bm, in1=c1_b, op=TT.mult)
                    nc.vector.tensor_scalar_mul(out=lnb[:], in0=lnb[:],
                                                scalar1=-0.5)
                    nc.vector.tensor_tensor(out=lnb[:], in0=lnb[:], in1=c0_b,
                                            op=TT.add)
                    nc.vector.tensor_tensor(out=bm, in0=bm, in1=lnb[:], op=TT.add)
                    nc.sync.dma_start(out=klb[tglob], in_=bia[:, :, 0])

                    lne = pool.tile([P, TC, e], F32, tag="lne")
                    nc.vector.tensor_tensor(out=es, in0=es, in1=es, op=TT.mult)
                    nc.scalar.activation(
                        out=lne[:], in_=es,
                        func=mybir.ActivationFunctionType.Ln)
                    nc.vector.tensor_tensor(out=em, in0=em, in1=mp_e,
                                            op=TT.subtract)
                    nc.vector.tensor_tensor(out=em, in0=em, in1=em, op=TT.mult)
                    nc.vector.tensor_tensor(out=em, in0=em, in1=es, op=TT.add)
                    nc.vector.tensor_tensor(out=em, in0=em, in1=c1_e, op=TT.mult)
                    nc.vector.tensor_scalar_mul(out=lne[:], in0=lne[:],
                                                scalar1=-0.5)
                    nc.vector.tensor_tensor(out=lne[:], in0=lne[:], in1=c0_e,
                                            op=TT.add)
                    nc.vector.tensor_tensor(out=em, in0=em, in1=lne[:], op=TT.add)
                    nc.sync.dma_start(out=kle[tglob], in_=em)
                    tglob += 1

            BC = BT // P
            for bt in range(ntb):
                b1 = tile_banks[bt]
                e0_t = pool.tile([P, BC, 64], F32, tag="e0")
                e1_t = pool.tile([P, BC, 64], F32, tag="e1t")
                for half in range(2):
                    r0 = half * CHUNK
                    col0 = r0 // P
                    i0 = (bt * BT + r0) // 16
                    nc.gpsimd.dma_gather(
                        out_ap=e0_t[:, col0:col0 + CHUNK // P, :],
                        in_ap=emb0[0:s0rows, :],
                        idxs_ap=idxB_t[:, i0:i0 + CHUNK // 16],
                        num_idxs=CHUNK, num_idxs_reg=CHUNK, elem_size=64,
                    )
                    i1 = (ntb * BT + bt * BT + r0) // 16
                    base = b1 * BANK
                    hi = min(base + BANK, s1rows)
                    nc.gpsimd.dma_gather(
                        out_ap=e1_t[:, col0:col0 + CHUNK // P, :],
                        in_ap=emb1[base:hi, :],
                        idxs_ap=idxB_t[:, i1:i1 + CHUNK // 16],
                        num_idxs=CHUNK, num_idxs_reg=CHUNK, elem_size=64,
                    )
                nc.vector.tensor_tensor(
                    out=e0_t[:, :, 0:34], in0=e0_t[:, :, 0:34],
                    in1=e1_t[:, :, 0:34], op=TT.mult)
                dot = pool.tile([P, BC], F32, tag="dot")
                nc.vector.reduce_sum(out=dot[:], in_=e0_t[:, :, 0:34],
                                     axis=mybir.AxisListType.X)
                nc.vector.tensor_tensor(
                    out=dot[:], in0=dot[:],
                    in1=mgb_t[:, 0:1].to_broadcast([P, BC]), op=TT.add)
                nc.sync.dma_start(out=pred[bt], in_=dot[:])

    nc.compile()
    return nc


_CACHE = {}
LAST_RESULTS = None


def _kl_normal_np(mq, sq, mp, sp):
    return (np.log(sp) - np.log(sq)
            + (sq**2 + (mq - mp) ** 2) / (2.0 * sp**2) - 0.5)


def _prepare(inputs):
    f32 = np.float32
    x0 = np.asarray(inputs["x0"]).astype(np.int64)
    x1 = np.asarray(inputs["x1"]).astype(np.int64)
    xu0 = np.asarray(inputs["xu0"]).astype(np.int64)
    xu1 = np.asarray(inputs["xu1"]).astype(np.int64)
    mgrbp = np.asarray(inputs["mean_group_bias_prior"], f32)
    sgrbp = np.asarray(inputs["scale_group_bias_prior"], f32)
    mgrep = np.asarray(inputs["mean_group_entity_prior"], f32)
    sgrep = np.asarray(inputs["scale_group_entity_prior"], f32)
    mgb = np.asarray(inputs["mean_global_bias"], f32)
    bias_params = np.ascontiguousarray(np.asarray(inputs["bias_params"], f32))
    entity_params = np.ascontiguousarray(
        np.asarray(inputs["entity_params"], f32))

    bp64 = np.zeros((NROWS, 64), f32)
    bp64[:, 0:2] = bias_params

    instrs0, s0rows = _schedule_s0()
    s1_idx16, instrs1, rank1 = _plan_s1(xu1)
    s1rows = len(s1_idx16)
    assert s0rows <= BANK

    # phase-B uniform tile->rank-bank assignment
    nb1 = -(-s1rows // BANK)
    bpc = B // N_CORES
    # count per (core, rank-bank), take max over cores, round to tiles
    owner = x0 // U1C
    rb = rank1[x1] // BANK
    tiles_per_bank = []
    for b in range(nb1):
        mx = 0
        for c in range(N_CORES):
            mx = max(mx, int(np.sum((owner == c) & (rb == b))))
        tiles_per_bank.append(-(-max(mx, 1) // BT))
    tile_banks = []
    for b in range(nb1):
        tile_banks += [b] * tiles_per_bank[b]
    ntb = len(tile_banks)

    baux = np.zeros((2, P, 3), f32)
    eaux = np.zeros((2, 3, P, 1, E), f32)
    for g in range(2):
        spb = abs(float(sgrbp[g]))
        baux[g, :, 0] = mgrbp[g]
        baux[g, :, 1] = np.log(f32(spb)) - f32(0.5)
        baux[g, :, 2] = 1.0 / (2.0 * f32(spb) ** 2)
        spe = np.abs(sgrep[g].astype(f32))
        eaux[g, 0, :, 0, :] = mgrep[g]
        eaux[g, 1, :, 0, :] = np.log(spe) - f32(0.5)
        eaux[g, 2, :, 0, :] = 1.0 / (2.0 * spe**2)
    mgb_tile = np.broadcast_to(mgb.reshape(1, 1), (P, 1)).copy()

    in_maps, metas = [], []
    s1w = wrap16(s1_idx16)
    for c in range(N_CORES):
        idx0_16, rank0 = _pack_s0(xu0[c * U1C:(c + 1) * U1C])
        idxA = np.concatenate([wrap16(idx0_16), s1w], axis=1)
        el = np.nonzero(owner == c)[0]
        el = el[np.argsort(rank1[x1[el]], kind="stable")]
        # distribute into per-bank tile slots; pads use idx 0 and are
        # excluded at assembly via el_counts
        el_tiles, el_counts = [], []
        bounds = np.searchsorted(rank1[x1[el]] // BANK, np.arange(nb1 + 1))
        for b in range(nb1):
            eb = el[bounds[b]:bounds[b + 1]]
            cap = tiles_per_bank[b] * BT
            assert len(eb) <= cap, (c, b, len(eb), cap)
            filler = eb[-1] if len(eb) else el[0]
            full = np.concatenate([eb, np.repeat(filler, cap - len(eb))])
            for t in range(tiles_per_bank[b]):
                lo = t * BT
                el_tiles.append(full[lo:lo + BT])
                el_counts.append(int(min(max(len(eb) - lo, 0), BT)))
        # idxB: [side0 all tiles | side1 all tiles]; pad slots -> idx 0
        i0 = np.empty((ntb, BT), np.int64)
        i1 = np.empty((ntb, BT), np.int64)
        for t, et in enumerate(el_tiles):
            n_t = el_counts[t]
            i0[t] = rank0[x0[et] - c * U1C]
            i1[t] = rank1[x1[et]] % BANK
            i0[t, n_t:] = 0
            i1[t, n_t:] = 0
        idxB = np.concatenate(
            [wrap16(i0.reshape(-1).astype(np.int16)),
             wrap16(i1.reshape(-1).astype(np.int16))], axis=1)
        in_maps.append(dict(e1=entity_params, bp64=bp64, idxA=idxA, idxB=idxB,
                            mgb=mgb_tile, baux=baux, eaux=eaux))
        metas.append((rank0, el_tiles, el_counts))
    return (in_maps, metas, rank1, s0rows, s1rows, instrs0, instrs1,
            tile_banks)


def bench_handles(inputs):
    """(nc, in_maps) for external timing harnesses."""
    (in_maps, metas, rank1, s0rows, s1rows, instrs0, instrs1,
     tile_banks) = _prepare(inputs)
    key = (s0rows, s1rows, len(tile_banks), tuple(tile_banks),
           tuple(map(tuple, instrs1)))
    if key not in _CACHE:
        _CACHE.clear()
        _CACHE[key] = build_nc_v2(N_CORES, s0rows, s1rows, instrs0, instrs1,
                                  tile_banks)
    return _CACHE[key], in_maps


def kernel(**inputs):
    f32 = np.float32
    (in_maps, metas, rank1, s0rows, s1rows, instrs0, instrs1,
     tile_banks) = _prepare(inputs)
    key = (s0rows, s1rows, len(tile_banks), tuple(tile_banks),
           tuple(map(tuple, instrs1)))
    if key not in _CACHE:
        _CACHE.clear()
        _CACHE[key] = build_nc_v2(N_CORES, s0rows, s1rows, instrs0, instrs1,
                                  tile_banks)
    nc = _CACHE[key]
    res = run_bass_kernel_spmd(nc, in_maps, core_ids=list(range(N_CORES)))
    global LAST_RESULTS
    LAST_RESULTS = res
    results = res.results

    x0 = np.asarray(inputs["x0"]).astype(np.int64)
    x1 = np.asarray(inputs["x1"]).astype(np.int64)
    alpha = np.asarray(inputs["alpha"], f32)
    mgb = np.asarray(inputs["mean_global_bias"], f32)
    sgb = np.asarray(inputs["scale_global_bias"], f32)
    mgbp = np.asarray(inputs["mean_global_bias_prior"], f32)
    sgbp = np.asarray(inputs["scale_global_bias_prior"], f32)

    nt0 = s0rows // TROWS
    nt1 = s1rows // TROWS
    kl_bias = np.empty(U1 + U2, f32)
    kl_entity = np.empty((U1 + U2, E), f32)
    pred = np.empty(B, f32)
    # stream-row r <-> (tile r//TROWS, partition r%128, col (r%TROWS)//128)
    for c in range(N_CORES):
        rank0, el_tiles, el_counts = metas[c]
        klb_c = results[c]["klb"]   # [nt0+nt1, 128, 32]
        kle_c = results[c]["kle"]
        flat_b0 = klb_c[:nt0].transpose(0, 2, 1).reshape(-1)
        flat_e0 = kle_c[:nt0].transpose(0, 2, 1, 3).reshape(-1, E)
        kl_bias[c * U1C:(c + 1) * U1C] = flat_b0[rank0]
        kl_entity[c * U1C:(c + 1) * U1C] = flat_e0[rank0]
        pred_c = results[c]["pred"]  # [ntb, 128, 16]
        flat_p = pred_c.transpose(0, 2, 1).reshape(len(el_tiles), -1)
        for t, et in enumerate(el_tiles):
            n_t = el_counts[t]
            pred[et[:n_t]] = flat_p[t][:n_t]
    klb_c0 = results[0]["klb"]
    kle_c0 = results[0]["kle"]
    flat_b1 = klb_c0[nt0:].transpose(0, 2, 1).reshape(-1)
    flat_e1 = kle_c0[nt0:].transpose(0, 2, 1, 3).reshape(-1, E)
    kl_bias[U1:] = flat_b1[rank1]
    kl_entity[U1:] = flat_e1[rank1]

    std = np.sqrt(1.0 / np.abs(alpha)).astype(f32)
    kl_global = _kl_normal_np(mgb, np.abs(sgb), mgbp, np.abs(sgbp)).astype(f32)
    return pred.astype(f32), std, kl_global, kl_bias, kl_entity
